# revision 2
# baseline (speedup 1.0000x reference)
"""ColAttention TRN2 kernel v3: 8-core data-parallel over batch (2/core).

Math per (batch, width-column w):
  Q = Wq@x+bq; K = Wk@x+bk; V = g*Wv@x (+bias folded via e-trick)
  S[h,g] = sum_q Q[q,h]K[q,g]; es = exp(S^T) [g,h]
  U = Wv'(x_col @ es) = g*Wv (x es)   (associativity: P = x@es first)
  out = U / colsum(es) + xb           (residual via e-trick, gamma in Wv')

Device pipeline (bf16 matmuls, f32 PSUM):
  host folds bv via e = g*(I+g*Wv)^-1 bv: xb = x+e, bq' = bq-Wq@e,
  bk' = bk-Wk@e; gamma folded into Wv' = g*Wv. Host also ships xT
  (h-partition transposed x) so the P matmul uses cheap K=96 stationaries.
  Per (batch, half):
    QK proj: lhsT=[WqT|WkT] packed M=128, N=512 chunks; bias-add evac DVE
    per 4-col chunk:
      S^T: 4 MM (lhsT=K_col[64,96], rhs=Q_col) -> s[96,384] PSUM
      exp: ACT -> es bf16
      colsum+bcast: 1 MM (lhsT=ones[96,128]) -> csb[128,384] replicated
      recip: DVE -> rb f32
      P: 8 MM (lhsT=xT[96,128] slices, rhs=es cols) -> [256,384] PSUM
      P evac: ACT -> bf16
      U: 4 MM (lhsT=WvT'[128,128] fixed, rhs=P slab N=384) -> u PSUM
      t = u*rb (DVE), f = t+xb (GpSimd)
    DMA f -> out
"""
import sys

sys.path.insert(0, "/opt/trn_rl_repo")

import numpy as np
import ml_dtypes

import concourse.bass as bass
import concourse.bacc as bacc
import concourse.mybir as mybir
import concourse.tile as tile
from concourse.bass_utils import run_bass_kernel_spmd


F32 = mybir.dt.float32
BF16 = mybir.dt.bfloat16
AF = mybir.ActivationFunctionType

P = 128
H = 96
W = 96
WH = 48          # columns per half
B_LOC = 2        # batches per core
WC = 4           # columns per chunk
NCH = WH // WC   # 12 chunks per half
NF = WC * H      # 384 free elements per chunk


def _build():
    nc = bacc.Bacc("TRN2", target_bir_lowering=False, debug=False)

    xb_d = nc.dram_tensor("xb", [B_LOC, 2, 2, P, H * WH], BF16, kind="ExternalInput")
    xt_d = nc.dram_tensor("xt", [B_LOC, 2, H, WH * 256], BF16, kind="ExternalInput")
    cb_d = nc.dram_tensor("cblob", [P, 896], BF16, kind="ExternalInput")
    bb_d = nc.dram_tensor("bblob", [P, 1], F32, kind="ExternalInput")
    out_d = nc.dram_tensor("out", [B_LOC, 2, 2, P, H * WH], BF16, kind="ExternalOutput")

    with tile.TileContext(nc) as tc:
        import contextlib

        ctx = contextlib.ExitStack()
        with ctx:
            consts = ctx.enter_context(tc.tile_pool(name="consts", bufs=1))
            xp = ctx.enter_context(tc.tile_pool(name="xp", bufs=2))
            xtp = ctx.enter_context(tc.tile_pool(name="xtp", bufs=2))
            qkp = ctx.enter_context(tc.tile_pool(name="qkp", bufs=2))
            esp = ctx.enter_context(tc.tile_pool(name="esp", bufs=2))
            rbp = ctx.enter_context(tc.tile_pool(name="rbp", bufs=2))
            psbp = ctx.enter_context(tc.tile_pool(name="psbp", bufs=2))
            ttp = ctx.enter_context(tc.tile_pool(name="ttp", bufs=2))
            fp = ctx.enter_context(tc.tile_pool(name="fp", bufs=2))
            ps = ctx.enter_context(tc.tile_pool(name="ps", bufs=1, space="PSUM"))

            cb_t = consts.tile([P, 896], BF16)
            bb_t = consts.tile([P, 1], F32)
            nc.sync.dma_start(out=cb_t, in_=cb_d.ap())
            nc.sync.dma_start(out=bb_t, in_=bb_d.ap())
            # observers: funnel DMA deps into one engine each
            nc.tensor.ldweights(cb_t[:, 0:128])
            bias_t = consts.tile([P, 1], F32)
            nc.vector.tensor_copy(bias_t, bb_t)

            qkw = [cb_t[:, ci * 128 : (ci + 1) * 128] for ci in range(2)]
            wv = [
                [cb_t[:, 256 + (cc * 2 + co) * 128 : 256 + (cc * 2 + co + 1) * 128]
                 for co in range(2)]
                for cc in range(2)
            ]
            ones_t = cb_t[0:H, 768:896]

            PC = 2304            # x/f piece size (6 add chunks)
            for b in range(B_LOC):
                for half in range(2):
                    x_ts = []
                    for pc in range(2):
                        x_pt = xp.tile([P, 2, PC], BF16, tag=f"x{pc}", name=f"xt_{pc}")
                        for ci in range(2):
                            nc.sync.dma_start(
                                out=x_pt[:, ci, :],
                                in_=xb_d.ap()[b, half, ci][:, pc * PC : (pc + 1) * PC],
                            )
                        x_ts.append(x_pt)
                    xt_ts = []
                    for pc in range(2):
                        xt_pt = xtp.tile([H, 6144], BF16, tag=f"xt{pc}", name=f"xtt_{pc}")
                        nc.sync.dma_start(
                            out=xt_pt,
                            in_=xt_d.ap()[b, half][:, pc * 6144 : (pc + 1) * 6144],
                        )
                        xt_ts.append(xt_pt)
                    f_ts = [fp.tile([P, 2, PC], BF16, tag=f"f{pc}", name=f"ft_{pc}")
                            for pc in range(2)]
                    q_t = qkp.tile([64, H * WH], BF16, tag="q")
                    k_t = qkp.tile([64, H * WH], BF16, tag="k")

                    # ---- QK projection, packed M=128 ----------------------
                    for nch in range(9):
                        pr = ps.tile([P, 512], F32, tag="qkpr", bufs=1)
                        # rhs may straddle the two x pieces
                        segs = []
                        start = nch * 512
                        while start < (nch + 1) * 512:
                            pc = start // PC
                            off = start - pc * PC
                            w = min((nch + 1) * 512 - start, PC - off)
                            segs.append((pc, off, start - nch * 512, w))
                            start += w
                        for (pc, off, oo, w) in segs:
                            for ci in range(2):
                                nc.tensor.matmul(
                                    pr[:, oo : oo + w], qkw[ci],
                                    x_ts[pc][:, ci, off : off + w],
                                    start=(ci == 0), stop=(ci == 1),
                                )
                        nc.vector.tensor_scalar(
                            out=q_t[:, nch * 512 : (nch + 1) * 512],
                            in0=pr[0:64, :], scalar1=bias_t[0:64, 0:1],
                            scalar2=None, op0=mybir.AluOpType.add,
                        )
                        nc.scalar.activation(
                            out=k_t[:, nch * 512 : (nch + 1) * 512],
                            in_=pr[64:128, :], func=AF.Identity,
                            bias=bias_t[64:128, 0:1], scale=1.0,
                        )

                    q_cols = q_t.rearrange("p (w h) -> p w h", h=H)
                    k_cols = k_t.rearrange("p (w h) -> p w h", h=H)

                    for ch in range(NCH):
                        # ---- scores S^T[g,h] per column -------------------
                        s_t = ps.tile([H, NF], F32, tag="s", bufs=1)
                        for j in range(WC):
                            wl = ch * WC + j
                            nc.tensor.matmul(
                                s_t[:, j * H : (j + 1) * H],
                                k_cols[:, wl, :], q_cols[:, wl, :],
                                start=True, stop=True,
                            )
                        es_t = esp.tile([H, NF], BF16, tag="es")
                        nc.scalar.activation(out=es_t, in_=s_t, func=AF.Exp)

                        # ---- colsum broadcast to 128 partitions -----------
                        csb = ps.tile([P, NF], F32, tag="csb", bufs=2)
                        nc.tensor.matmul(csb, ones_t, es_t, start=True, stop=True)
                        rb_t = rbp.tile([P, NF], F32, tag="rb")
                        nc.vector.reciprocal_approx_fast(out=rb_t, in_=csb)

                        # ---- P = x_col @ es (per col, xT stationary) ------
                        tpc, toff0 = divmod(ch * WC * 256, 6144)
                        pps = [ps.tile([P, NF], F32, tag=f"p{cc}", bufs=1, name=f"pp{cc}") for cc in range(2)]
                        for j in range(WC):
                            toff = toff0 + j * 256
                            for cc in range(2):
                                nc.tensor.matmul(
                                    pps[cc][:, j * H : (j + 1) * H],
                                    xt_ts[tpc][:, toff + cc * 128 : toff + (cc + 1) * 128],
                                    es_t[:, j * H : (j + 1) * H],
                                    start=True, stop=True,
                                )
                        p_sb = [psbp.tile([P, NF], BF16, tag=f"ps{cc}", name=f"psb{cc}") for cc in range(2)]
                        for cc in range(2):
                            nc.vector.tensor_mul(p_sb[cc], pps[cc], rb_t)

                        # ---- U = Wv' @ (P*rb), residual -------------------
                        for co in range(2):
                            u_t = ps.tile([P, NF], F32, tag=f"u{co}", bufs=1, name=f"ut{co}")
                            for cc in range(2):
                                nc.tensor.matmul(
                                    u_t, wv[cc][co], p_sb[cc],
                                    start=(cc == 0), stop=(cc == 1),
                                )
                            t_t = ttp.tile([P, NF], BF16, tag=f"t{co}", name=f"tt{co}")
                            nc.scalar.copy(out=t_t, in_=u_t)
                            xpc, xoff = divmod(ch * NF, PC)
                            xsl = x_ts[xpc][:, co, xoff : xoff + NF]
                            fsl = f_ts[xpc][:, co, xoff : xoff + NF]
                            nc.gpsimd.tensor_add(fsl, t_t, xsl)

                        if ch % 6 == 5:
                            xpc = ch // 6
                            for ci in range(2):
                                nc.sync.dma_start(
                                    out=out_d.ap()[b, half, ci][:, xpc * PC : (xpc + 1) * PC],
                                    in_=f_ts[xpc][:, ci, :],
                                )
    nc.compile()
    return nc


_NC_CACHE = None


def _get_nc():
    global _NC_CACHE
    if _NC_CACHE is None:
        _NC_CACHE = _build()
    return _NC_CACHE


def _host_prep(x, Wq, bq, Wk, bk, Wv, bv, gamma):
    x = np.asarray(x, np.float32)
    Wq = np.asarray(Wq, np.float32)
    bq = np.asarray(bq, np.float32)
    Wk = np.asarray(Wk, np.float32)
    bk = np.asarray(bk, np.float32)
    Wv = np.asarray(Wv, np.float32)
    bv = np.asarray(bv, np.float32)
    g = float(np.asarray(gamma, np.float32)[0])

    C = 256
    e = (g * np.linalg.solve(np.eye(C, dtype=np.float64) + g * Wv.astype(np.float64),
                             bv.astype(np.float64))).astype(np.float32)
    xb_f = x + e[None, :, None, None]
    # xb: [16, half, ci, 128, h*48] h-major within half
    xb = (
        xb_f.reshape(16, 2, P, H, 2, WH)
        .transpose(0, 4, 1, 2, 5, 3)
        .reshape(16, 2, 2, P, H * WH)
        .astype(ml_dtypes.bfloat16)
    )
    xb = np.ascontiguousarray(xb)
    # xT: [16, half, h, wl*256 + c]
    xt = (
        xb_f.transpose(0, 2, 3, 1)
        .reshape(16, H, 2, WH, C)
        .transpose(0, 2, 1, 3, 4)
        .reshape(16, 2, H, WH * C)
        .astype(ml_dtypes.bfloat16)
    )
    xt = np.ascontiguousarray(xt)

    wvg = (g * Wv).astype(np.float32)
    cblob = np.zeros((P, 896), np.float32)
    for ci in range(2):
        cblob[:, ci * 128 : ci * 128 + 64] = Wq[:, ci * 128 : (ci + 1) * 128].T
        cblob[:, ci * 128 + 64 : (ci + 1) * 128] = Wk[:, ci * 128 : (ci + 1) * 128].T
    wvgT = wvg.T  # [c_in, c_out]
    for cc in range(2):
        for co in range(2):
            cblob[:, 256 + (cc * 2 + co) * 128 : 256 + (cc * 2 + co + 1) * 128] = (
                wvgT[cc * 128 : (cc + 1) * 128, co * 128 : (co + 1) * 128]
            )
    cblob[0:H, 768:896] = 1.0
    cblob = cblob.astype(ml_dtypes.bfloat16)

    bblob = np.zeros((P, 1), np.float32)
    bblob[0:64, 0] = bq - Wq @ e
    bblob[64:128, 0] = bk - Wk @ e

    in_maps = []
    for core in range(8):
        in_maps.append({
            "xb": xb[core * B_LOC : (core + 1) * B_LOC],
            "xt": xt[core * B_LOC : (core + 1) * B_LOC],
            "cblob": cblob, "bblob": bblob,
        })
    return in_maps


def _unshard(results):
    outs = []
    for r in results:
        a = r["out"].astype(np.float32).reshape(B_LOC, 2, 2, P, WH, H)
        outs.append(a.transpose(0, 2, 3, 5, 1, 4).reshape(B_LOC, 256, H, W))
    return np.concatenate(outs, axis=0)


def kernel(x, Wq, bq, Wk, bk, Wv, bv, gamma):
    in_maps = _host_prep(x, Wq, bq, Wk, bk, Wv, bv, gamma)
    nc = _get_nc()
    res = run_bass_kernel_spmd(nc, in_maps, core_ids=list(range(8)))
    return _unshard(res.results)


def prepared_in_maps(inputs):
    """test-harness helper: the per-core in_maps for a full input dict."""
    return _host_prep(**inputs)
